# revision 1
# baseline (speedup 1.0000x reference)
"""Trainium2 Bass kernel for nn_Attention (dense_transformer).

Reference computation (per batch b of 4, dim C=256, HEADS=4, hc=64, N=48*48=2304):
  k = wk@x+bk; q = wq@x+bq; v = wv@x+bv          (1x1 convs = channel GEMMs)
  dots[n,m] = sum_c k[c,n] q[c,m]   per head
  attn = softmax(dots, axis=keys n)
  out  = v @ attn ; y = wo@out + bo

Sharding: 8 cores, core c -> (batch c//2, query-half c%2). Each core computes
all 4 heads for its 1152 queries; keys are always the full 2304 positions.
No collectives needed; host reassembles by pure concatenation.

Algebraic folds used on device:
  - bk cancels in softmax over keys (adds a per-query constant to dots).
  - bq folded into q during the PSUM->SBUF copy (per-partition scalar add).
  - bv folded at the end:   y = wo@(att) + (wo@bv + bo)  since sum_n attn = 1.
  - softmax normalizer: ones column appended to v^T so the attn@v matmul
    also produces sumexp; normalization is a per-query reciprocal + broadcast.
"""

import numpy as np

from concourse import bacc, bass, mybir, tile



F32 = mybir.dt.float32
F32R = mybir.dt.float32r
BF16 = mybir.dt.bfloat16

B, C, HW, HEADS, HC = 4, 256, 48, 4, 64
N = HW * HW          # 2304 keys
M = N // 2           # 1152 queries per core
NT = N // 128        # 18 key tiles
WIN = 1536           # exp window (3 PSUM banks)
FLAT = NT * M        # 20736 flat dots cols per head

_CACHED = {}
last_in_maps = None


def _chunks(total, start_align=0):
    """Split [0,total) at multiples of 512 of (start_align + offset)."""
    out = []
    pos = 0
    while pos < total:
        nxt = min(total, ((start_align + pos) // 512 + 1) * 512 - start_align)
        out.append((pos, nxt - pos))
        pos = nxt
    return out


def build_nc():
    nc = bacc.Bacc("TRN2", target_bir_lowering=False, debug=False)

    x_d = nc.dram_tensor("x", [C, N], F32R, kind="ExternalInput")
    xq_d = nc.dram_tensor("xq", [C, M], F32R, kind="ExternalInput")
    w3_d = nc.dram_tensor("w3", [C, 3 * C], F32R, kind="ExternalInput")
    woT_d = nc.dram_tensor("woT", [C, C], BF16, kind="ExternalInput")
    bias_d = nc.dram_tensor("bias", [128, 8], F32, kind="ExternalInput")
    y_d = nc.dram_tensor("out", [C, M], F32, kind="ExternalOutput")

    sb = lambda name, shape, dt: nc.alloc_sbuf_tensor(name, shape, dt).ap()

    x_sb = [sb(f"x{i}", [128, N], F32R) for i in range(2)]
    xq_sb = [sb(f"xq{i}", [128, M], F32R) for i in range(2)]
    w3_sb = [sb(f"w3_{i}", [128, 3 * C], F32R) for i in range(2)]
    wkT = [w3_sb[i][:, 0:C] for i in range(2)]
    wqT = [w3_sb[i][:, C:2 * C] for i in range(2)]
    wvT = [w3_sb[i][:, 2 * C:3 * C] for i in range(2)]
    woT = [sb(f"woT{i}", [128, C], BF16) for i in range(2)]
    bias_sb = sb("bias_sb", [128, 8], F32)
    bqd_sb = bias_sb[:, 0:4]
    bv_sb = bias_sb[:, 4:6]
    bo_sb = bias_sb[:, 6:8]
    # k/q duplicated into both partition halves so dots can row-group-pack
    # (even exp-windows use rows 0:64, odd use rows 64:128; adjacent windows'
    # matmuls run concurrently in different PE row groups)
    k2 = [sb(f"k2_{h}", [128, N], BF16) for h in range(HEADS)]
    qb2 = [sb(f"qb2_{h}", [128, M], BF16) for h in range(HEADS)]
    vT_all = sb("vT_all", [128, NT * (HC + 1) * HEADS], BF16)
    VS = (HC + 1) * HEADS
    vT = [vT_all[:, t * VS:(t + 1) * VS] for t in range(NT)]
    # exp output, one tensor per (buffer, m-chunk block) to keep the
    # dots->exp stream free of false WAR deps against attnv reads
    a_blk = [[sb(f"a{i}_{b}", [128, w], BF16) for b, w in
              enumerate([9216, 9216, 2304])] for i in range(2)]
    att = [sb(f"att{i}", [128, M], BF16) for i in range(2)]
    y_sb = [sb(f"y{i}", [128, M], F32) for i in range(2)]
    rcp2 = [sb(f"rcp{i}", [1, M], F32) for i in range(2)]
    bcast2 = [sb(f"bcast{i}", [64, 512], F32) for i in range(2)]
    fb = sb("fb", [128, 2], F32)

    MC = _chunks(M)  # [(0,512),(512,512),(1024,128)]

    with tile.TileContext(nc) as tc, nc.allow_low_precision(
            reason="attention weights; rel-err budget 2e-2"):
        with (
            tc.tile_pool(name="ps_dots", bufs=2, space="PSUM") as dpool,
            tc.tile_pool(name="ps_acc", bufs=2, space="PSUM") as apool,
        ):
            # ---- input DMAs ----
            # SP ring: bias + k/q weights, then it is free for the k2/qb2
            # duplication DMAs emitted by the projections.  ACT ring: first
            # x piece, xq (query slice), remaining x, then v/o weights.
            nc.sync.dma_start(bias_sb[:, :], bias_d.ap()[:, :])
            for i in range(2):
                cs = slice(i * 128, (i + 1) * 128)
                nc.sync.dma_start(w3_sb[i][:, 0:2 * C], w3_d.ap()[cs, 0:2 * C])
            for i in range(2):
                cs = slice(i * 128, (i + 1) * 128)
                nc.scalar.dma_start(x_sb[i][:, 0:768], x_d.ap()[cs, 0:768])
            for i in range(2):
                cs = slice(i * 128, (i + 1) * 128)
                nc.scalar.dma_start(xq_sb[i][:, :], xq_d.ap()[cs, :])
            for (c0, cw) in [(768, 768), (1536, 768)]:
                for i in range(2):
                    cs = slice(i * 128, (i + 1) * 128)
                    nc.scalar.dma_start(
                        x_sb[i][:, c0:c0 + cw], x_d.ap()[cs, c0:c0 + cw])
            for i in range(2):
                cs = slice(i * 128, (i + 1) * 128)
                nc.scalar.dma_start(
                    w3_sb[i][:, 2 * C:3 * C], w3_d.ap()[cs, 2 * C:3 * C])
                nc.scalar.dma_start(woT[i][:, :], woT_d.ap()[cs, :])
            # warm the ACT exp table while DMAs/projections run
            warm = sb("warm", [1, 2], F32)
            nc.vector.memset(warm[:, :], 0.0)
            nc.scalar.activation(warm[:, :], warm[:, :],
                                 mybir.ActivationFunctionType.Exp)

            # ---- fb = wo@bv + bo (per o-tile column; bf16 matmul) ----
            bv_bf = sb("bv_bf", [128, 2], BF16)
            nc.vector.tensor_copy(bv_bf[:, :], bv_sb[:, :])
            for ot in range(2):
                ps = apool.tile([128, 512], F32, tag="acc")
                for ct in range(2):
                    nc.tensor.matmul(
                        ps[:, 0:1],
                        woT[ct][:, ot * 128:(ot + 1) * 128],
                        bv_bf[:, ct:ct + 1],
                        start=(ct == 0), stop=(ct == 1),
                    )
                nc.vector.tensor_add(fb[:, ot:ot + 1], ps[:, 0:1], bo_sb[:, ot:ot + 1])

            # ---- k/q projections (pair matmuls + per-chunk dup DMAs) ----
            # Each head's k/q is duplicated into both partition halves so
            # dots can row-group-pack by exp-window parity.
            lo, hi = slice(0, 64), slice(64, 128)

            def emit_proj_kq(mt):
                h0, h1 = 2 * mt, 2 * mt + 1
                for (c0, cw) in _chunks(N):
                    ps = apool.tile([128, 512], F32, tag="acc")
                    for ct in range(2):
                        nc.tensor.matmul(
                            ps[:, 0:cw],
                            wkT[ct][:, mt * 128:(mt + 1) * 128],
                            x_sb[ct][:, c0:c0 + cw],
                            start=(ct == 0), stop=(ct == 1),
                        )
                    cs = slice(c0, c0 + cw)
                    nc.vector.tensor_copy(k2[h0][lo, cs], ps[0:64, 0:cw])
                    nc.vector.tensor_copy(k2[h1][hi, cs], ps[64:128, 0:cw])
                    nc.sync.dma_start(k2[h0][hi, cs], k2[h0][lo, cs])
                    nc.sync.dma_start(k2[h1][lo, cs], k2[h1][hi, cs])
                for (c0, cw) in MC:
                    ps = apool.tile([128, 512], F32, tag="acc")
                    for ct in range(2):
                        nc.tensor.matmul(
                            ps[:, 0:cw],
                            wqT[ct][:, mt * 128:(mt + 1) * 128],
                            xq_sb[ct][:, c0:c0 + cw],
                            start=(ct == 0), stop=(ct == 1),
                        )
                    cs = slice(c0, c0 + cw)
                    nc.vector.tensor_scalar_add(
                        qb2[h0][lo, cs], ps[0:64, 0:cw], bqd_sb[0:64, h0:h0 + 1])
                    nc.vector.tensor_scalar_add(
                        qb2[h1][hi, cs], ps[64:128, 0:cw], bqd_sb[64:128, h1:h1 + 1])
                    nc.sync.dma_start(qb2[h0][hi, cs], qb2[h0][lo, cs])
                    nc.sync.dma_start(qb2[h1][lo, cs], qb2[h1][hi, cs])

            emit_proj_kq(0)

            # ---- v^T projection (+ ones column per head for sumexp) ----
            def emit_proj_vT():
                nc.vector.memset(vT_all[:, :], 1.0)
                for t in range(NT):
                    ps = apool.tile([128, 512], F32, tag="acc")
                    for ct in range(2):
                        nc.tensor.matmul(
                            ps[:, 0:C],
                            x_sb[ct][:, t * 128:(t + 1) * 128],
                            wvT[ct][:, :],
                            start=(ct == 0), stop=(ct == 1),
                        )
                    dst = vT[t][:].rearrange("p (h c) -> p h c", c=HC + 1)[:, :, 0:HC]
                    src_ = ps[:, 0:C].rearrange("p (h c) -> p h c", c=HC)
                    nc.vector.tensor_copy(dst, src_)

            # ---- per-head attention, software-pipelined emission ----
            # Flat dots layout is m-chunk-major: col(t, m in chunk b) =
            # BASE[b] + t*MW[b] + (m - M0[b]).  Group Gk(h) = the exp windows
            # of m-chunk k; B(h,k) (attnv+normalize) is ready after Gk(h).
            # Emission order staggers B one group behind the dots stream so
            # the PE always prefers feeding ACT's next exp window.
            BLK = [(0, 512, 0), (512, 512, 9216), (1024, 128, 18432)]
            WIN_OF = [(0, 6), (6, 12), (12, 14)]   # window range per m-chunk
            nwin = (FLAT + WIN - 1) // WIN

            def win_mms(j):
                w0, w1 = j * WIN, min(FLAT, (j + 1) * WIN)
                out = []
                for (m0, mw, base) in BLK:
                    for t in range(NT):
                        c0 = base + t * mw
                        if c0 < w1 and c0 + mw > w0:
                            assert c0 >= w0 and c0 + mw <= w1
                            out.append((t, m0, mw, c0 - w0))
                return out

            def emit_A_group(h, g):
                a = a_blk[h % 2][g]
                base_g = BLK[g][2]
                # head 0's first windows run while the k2/qb2 duplication
                # DMAs are still queued behind the input stream; use the
                # native (even) rows there so they don't wait on the dups --
                # the PE is idle at that point, so the lost row-group
                # concurrency is free.
                native_only = h == 0 and g <= 1
                for j in range(*WIN_OF[g]):
                    w0 = j * WIN
                    wlen = min(WIN, FLAT - w0)
                    D = dpool.tile([128, WIN], F32, tag="dots")
                    rows = slice(0, 64) if (j % 2 == 0 or native_only) \
                        else slice(64, 128)
                    for (t, mm0, mmw, doff) in win_mms(j):
                        nc.tensor.matmul(
                            D[:, doff:doff + mmw],
                            k2[h][rows, t * 128:(t + 1) * 128],
                            qb2[h][rows, mm0:mm0 + mmw],
                            start=True, stop=True,
                            tile_position=(rows.start, 0),
                        )
                    nc.scalar.activation(
                        a[:, w0 - base_g:w0 - base_g + wlen], D[:, 0:wlen],
                        mybir.ActivationFunctionType.Exp)

            def emit_B_chunk(h, bi):
                a = a_blk[h % 2][bi]
                m0, mw, base = BLK[bi]
                p = (3 * h + bi) % 2
                rcp, bcast = rcp2[p], bcast2[p]
                o2 = apool.tile([128, 512], F32, tag="acc")
                for t in range(NT):
                    nc.tensor.matmul(
                        o2[0:HC + 1, 0:mw],
                        vT[t][:, h * (HC + 1):(h + 1) * (HC + 1)],
                        a[:, t * mw:t * mw + mw],
                        start=(t == 0), stop=(t == NT - 1),
                    )
                nc.vector.reciprocal(rcp[0:1, m0:m0 + mw], o2[HC:HC + 1, 0:mw])
                nc.gpsimd.partition_broadcast(
                    bcast[:, 0:mw], rcp[0:1, m0:m0 + mw])
                nc.vector.tensor_mul(
                    att[h // 2][(h % 2) * 64:(h % 2) * 64 + 64, m0:m0 + mw],
                    o2[0:HC, 0:mw], bcast[:, 0:mw])
                if h == HEADS - 1:
                    emit_unify(bi)

            def emit_unify(bi):
                m0, mw, _ = BLK[bi]
                for ot in range(2):
                    u = apool.tile([128, 512], F32, tag="acc")
                    for ct in range(2):
                        nc.tensor.matmul(
                            u[:, 0:mw],
                            woT[ct][:, ot * 128:(ot + 1) * 128],
                            att[ct][:, m0:m0 + mw],
                            start=(ct == 0), stop=(ct == 1),
                        )
                    nc.vector.tensor_scalar_add(
                        y_sb[ot][:, m0:m0 + mw], u[:, 0:mw], fb[:, ot:ot + 1])
                    nc.sync.dma_start(
                        y_d.ap()[ot * 128:(ot + 1) * 128, m0:m0 + mw],
                        y_sb[ot][:, m0:m0 + mw])

            groups = [(h, g) for h in range(HEADS) for g in range(3)]
            emit_A_group(0, 0)
            emit_proj_kq(1)
            emit_A_group(0, 1)
            emit_proj_vT()
            LAG = 3
            for i in range(2, len(groups)):
                emit_A_group(*groups[i])
                if i >= LAG:
                    emit_B_chunk(*groups[i - LAG])
            for j in range(LAG, 0, -1):
                emit_B_chunk(*groups[-j])

    nc.compile()
    return nc


def _get_nc():
    if "nc" not in _CACHED:
        _CACHED["nc"] = build_nc()
    return _CACHED["nc"]


def kernel(x, wk, bk, wq, bq, wv, bv, wo, bo):
    from concourse import bass_utils

    import ml_dtypes
    bf16 = ml_dtypes.bfloat16
    x = np.ascontiguousarray(np.asarray(x, dtype=np.float32))
    mk = lambda w: np.ascontiguousarray(np.asarray(w, dtype=np.float32).T)
    w3 = np.ascontiguousarray(np.concatenate([mk(wk), mk(wq), mk(wv)], axis=1))
    woT = mk(wo).astype(bf16)
    col2 = lambda b: np.asarray(b, dtype=np.float32).reshape(2, 128).T
    bqd = np.asarray(bq, dtype=np.float32).reshape(4, 64)
    bqd = np.concatenate([bqd, bqd], axis=1).T          # [128, 4], both halves
    bias = np.ascontiguousarray(
        np.concatenate([bqd, col2(bv), col2(bo)], axis=1))

    xb = x.reshape(B, C, N)

    nc = _get_nc()
    in_maps = []
    for c in range(8):
        b, qh = c // 2, c % 2
        in_maps.append({
            "x": xb[b],
            "xq": np.ascontiguousarray(xb[b][:, qh * M:(qh + 1) * M]),
            "w3": w3, "woT": woT, "bias": bias,
        })
    global last_in_maps
    last_in_maps = in_maps
    res = bass_utils.run_bass_kernel_spmd(nc, in_maps, core_ids=list(range(8)))

    out = np.empty((B, C, N), dtype=np.float32)
    for c in range(8):
        b, qh = c // 2, c % 2
        out[b][:, qh * M:(qh + 1) * M] = res.results[c]["out"]
    return out.reshape(B, C, HW, HW)



# revision 32
# speedup vs baseline: 1.2565x; 1.2565x over previous
"""Trainium2 Bass kernel for nn_Attention (dense_transformer).

Reference (per batch b of 4, dim C=256, HEADS=4, hc=64, N=48*48=2304):
  k = wk@x+bk; q = wq@x+bq; v = wv@x+bv          (1x1 convs = channel GEMMs)
  dots[n,m] = sum_c k[c,n] q[c,m]   per head
  attn = softmax(dots, axis=keys n)
  out  = v @ attn ; y = wo@out + bo

Sharding: 8 cores, core c -> (batch c//2, query-half c%2). Each core computes
all 4 heads for its 1152 queries against all 2304 keys. No collectives.

Design notes (engine-balance driven):
  - exp(dots) can only run on the ACT engine: 82,944 psum columns is the hard
    floor (~80us busy).  Everything else is arranged so ACT streams exp
    windows back-to-back from ~5us on.
  - attn@v runs TRANSPOSED: stationary = exp-weight tile [128 keys, 128 q],
    moving = v^T columns (64 + ones col for sumexp) -> 42k PE cycles instead
    of 83k, giving PE the slack to always stay ahead of ACT.
  - All inputs bf16 (halves DMA, keeps every matmul at 1 cycle/col).
  - bk cancels in softmax over keys; bq folded into the q psum->sbuf copy;
    bv folded via fb = wo@bv + bo (host-precomputed weight constant since
    sum_n attn = 1); sumexp via a ones column appended to v^T.
  - Normalization in the transposed layout is a per-partition scalar multiply
    (queries = partitions), so no gpsimd broadcast is needed.
  - DMAs issue from the otherwise-idle SP ring (HWDGE).  Dummy matmuls at
    t~0 ramp the PE p-state before the first projection.
  - Per-head "block" order (which (mtile, keytile) dots tile goes where in
    the flat exp stream) is a free parameter; head 0 defers the late
    keytiles of mtiles 0-1 so the first windows only need the first x DMAs.
"""

import numpy as np

from concourse import bacc, bass, mybir, tile


F32 = mybir.dt.float32
BF16 = mybir.dt.bfloat16

B, C, HW, HEADS, HC = 4, 256, 48, 4, 64
N = HW * HW          # 2304 keys
M = N // 2           # 1152 queries per core
NT = N // 128        # 18 key tiles
NJ = M // 128        # 9 query tiles (mtiles)
BLK = NT * 128       # 2304 flat cols per mtile
FLAT = NJ * BLK      # 20736 flat dots cols per head
WIN = 1536           # dots psum tile (3 PSUM banks); windows may use less
VS = HC + 1          # v^T cols per head incl. ones col
N_WARMUP = 52        # PE p-state warm-up matmuls (end ~= first x chunk ready)

KCH = [(0, 512), (512, 512), (1024, 512), (1536, 512), (2048, 256)]
QCH = [(0, 256), (256, 256), (512, 512), (1024, 128)]   # j0+j1 split out

# block order per head: list of (j, t) pairs; position in list = position of
# that [128 keys x 128 queries] dots tile in the flat exp stream.  Head 0's
# prefix is ordered (and its first three windows shrunk to 8 blocks) so each
# window only needs x/xq DMA pieces that have already arrived.
_STD = [(j, t) for j in range(NJ) for t in range(NT)]
_H0 = ([(0, t) for t in range(4)] + [(1, t) for t in range(4)]          # w0
       + [(0, t) for t in range(4, 8)] + [(1, t) for t in range(4, 8)]  # w1
       + [(0, t) for t in range(8, 12)] + [(1, t) for t in range(8, 12)]  # w2
       + [(2, t) for t in range(12)]                                    # w3
       + [(0, t) for t in range(12, 18)] + [(1, t) for t in range(12, 18)]  # w4
       + [(2, t) for t in range(12, 18)] + [(3, t) for t in range(6)]   # w5
       + [(3, t) for t in range(6, 18)]                                 # w6
       + [(j, t) for j in range(4, NJ) for t in range(NT)])
BLOCKS = [_H0, _STD, _STD, _STD]
POS = [{jt: i for i, jt in enumerate(bl)} for bl in BLOCKS]
# window sizes (in 128-col blocks) per head; last windows kept >=9 blocks so
# the exp of the final window leaves PE enough budget across head transitions
_WS0 = [8, 8, 8] + [12] * 10 + [9, 9]
_WSS = [12] * 12 + [9, 9]
WINSZ = [_WS0, _WSS, _WSS, _WSS]
# WINDOWS[h] = list of (first_block_index, n_blocks)
WINDOWS = []
for ws in WINSZ:
    offs, o = [], 0
    for n in ws:
        offs.append((o, n))
        o += n
    assert o == NJ * NT
    WINDOWS.append(offs)
# window at which mtile j's last dots tile is produced, per head
READY = []
for h, bl in enumerate(BLOCKS):
    last = {}
    for i, (j, t) in enumerate(bl):
        last[j] = next(w for w, (o, n) in enumerate(WINDOWS[h]) if o <= i < o + n)
    READY.append(last)

_CACHED = {}
last_in_maps = None


def build_nc():
    nc = bacc.Bacc("TRN2", target_bir_lowering=False, debug=False)

    x_d = nc.dram_tensor("x", [128, 2 * N], BF16, kind="ExternalInput")
    xq_d = nc.dram_tensor("xq", [128, 2 * M], BF16, kind="ExternalInput")
    w3_d = nc.dram_tensor("w3", [128, 6 * C], BF16, kind="ExternalInput")
    woT_d = nc.dram_tensor("woT", [128, 2 * C], BF16, kind="ExternalInput")
    bias_d = nc.dram_tensor("bias", [128, 4], F32, kind="ExternalInput")
    y_d = nc.dram_tensor("out", [C, M], F32, kind="ExternalOutput")

    sb = lambda name, shape, dt: nc.alloc_sbuf_tensor(name, shape, dt).ap()

    x_sb = sb("x_sb", [128, 2 * N], BF16)
    xq_sb = sb("xq_sb", [128, 2 * M], BF16)
    w3_sb = sb("w3_sb", [128, 6 * C], BF16)
    woT_sb = sb("woT_sb", [128, 2 * C], BF16)
    bias_sb = sb("bias_sb", [128, 4], F32)
    k2 = [sb(f"k2_{mt}", [128, N], BF16) for mt in range(2)]
    qb2 = [sb(f"qb2_{mt}", [128, M], BF16) for mt in range(2)]
    vT = sb("vT", [128, NT * VS * HEADS], BF16)
    a2 = [sb(f"a{i}", [128, FLAT], BF16) for i in range(2)]
    attT = sb("attT", [128, NJ * 256], BF16)
    att = sb("att", [128, 2 * M], BF16)
    rcp = sb("rcp", [128, 2], F32)
    y_sb = sb("y_sb", [128, 2 * M], F32)
    ident = sb("ident", [128, 128], BF16)
    scratch = sb("scratch", [128, 128], BF16)
    warm = sb("warm", [1, 2], F32)

    # sbuf slice helpers
    def x_sl(c, ct):
        c0, cw = KCH[c]
        base = 1024 * c if c < 4 else 4096
        return x_sb[:, base + ct * cw: base + (ct + 1) * cw]

    def xv_sl(t, ct):
        c = t // 4
        cw = KCH[c][1]
        base = 1024 * c if c < 4 else 4096
        o = base + ct * cw + 128 * (t % 4)
        return x_sb[:, o: o + 128]

    def xq_sl(qc, ct):
        q0, qw = QCH[qc]
        pbase, pq0, pcw = (0, 0, 512) if q0 < 512 else \
                          (1024, 512, 512) if q0 < 1024 else (2048, 1024, 128)
        o = pbase + ct * pcw + (q0 - pq0)
        return xq_sb[:, o: o + qw]

    def xq_dma(q0, qw):      # strided piece covering both ct halves
        v = xq_d.ap()[:, 0:1024].rearrange("p (ct m) -> p ct m", ct=2)
        s = xq_sb[:, 0:1024].rearrange("p (ct m) -> p ct m", ct=2)
        nc.sync.dma_start(s[:, :, q0:q0 + qw], v[:, :, q0:q0 + qw])

    wk_sl = lambda mt, ct: w3_sb[:, ct * 768 + mt * 128: ct * 768 + (mt + 1) * 128]
    wq_sl = lambda mt, ct: w3_sb[:, ct * 768 + 256 + mt * 128: ct * 768 + 256 + (mt + 1) * 128]
    wv_sl = lambda ct: w3_sb[:, ct * 768 + 512: ct * 768 + 768]
    wo_sl = lambda ct, ot: woT_sb[:, ct * 256 + ot * 128: ct * 256 + (ot + 1) * 128]

    with tile.TileContext(nc) as tc, nc.allow_low_precision(
            reason="attention weights; rel-err budget 2e-2"):
        with (
            tc.tile_pool(name="ps_dots", bufs=2, space="PSUM") as dpool,
            tc.tile_pool(name="ps_acc", bufs=2, space="PSUM") as apool,
        ):
            # ---- input DMAs on the idle SP ring, critical pieces first ----
            # (a consumer sees DMA data ~900ns after transfer end: sem prop)
            w3v = w3_d.ap()[:].rearrange("p (ct q) -> p ct q", ct=2)
            w3s = w3_sb[:].rearrange("p (ct q) -> p ct q", ct=2)
            dma = nc.sync.dma_start
            dma(w3s[:, :, 0:128], w3v[:, :, 0:128])          # wk mt0
            dma(x_sb[:, 0:1024], x_d.ap()[:, 0:1024])        # x chunk 0
            dma(x_sb[:, 1024:2048], x_d.ap()[:, 1024:2048])  # x chunk 1
            dma(bias_sb[:, :], bias_d.ap()[:, :])
            dma(w3s[:, :, 256:384], w3v[:, :, 256:384])      # wq mt0
            xq_dma(0, 256)                                   # q cols of j0+j1
            dma(x_sb[:, 2048:3072], x_d.ap()[:, 2048:3072])  # x chunk 2
            xq_dma(256, 256)                                 # q cols j2+j3
            dma(x_sb[:, 3072:4096], x_d.ap()[:, 3072:4096])  # x chunk 3
            dma(x_sb[:, 4096:4608], x_d.ap()[:, 4096:4608])  # x chunk 4
            dma(w3s[:, :, 128:256], w3v[:, :, 128:256])      # wk mt1
            dma(w3s[:, :, 384:768], w3v[:, :, 384:768])      # wq mt1 + wv
            dma(xq_sb[:, 1024:2304], xq_d.ap()[:, 1024:2304])
            dma(woT_sb[:, :], woT_d.ap()[:, :])

            # warm the ACT exp table while DMAs/warm-up matmuls run
            nc.vector.memset(warm[:, :], 0.0)
            nc.scalar.activation(warm[:, :], warm[:, :],
                                 mybir.ActivationFunctionType.Exp)

            # PE p-state warm-up (full speed needs ~3us of continuous PE work)
            nc.vector.memset(scratch[:, :], 0.0)
            dtile = apool.tile([128, 512], F32, tag="acc")
            for _ in range(N_WARMUP):
                nc.tensor.matmul(dtile[:, 0:64], scratch[:, 0:128],
                                 scratch[:, 0:64], start=True, stop=True)

            # gpsimd setup (Pool is otherwise idle)
            from concourse.masks import make_identity
            vT4 = vT[:].rearrange("p (t h s) -> p t h s", t=NT, h=HEADS)
            nc.gpsimd.memset(vT4[:, :, :, HC:VS], 1.0)
            make_identity(nc, ident[:, :])

            # ---- emit helpers ----
            def emit_kproj(mt, c, on_act=False):
                c0, cw = KCH[c]
                ps = apool.tile([128, 512], F32, tag="acc")
                for ct in range(2):
                    nc.tensor.matmul(ps[:, 0:cw], wk_sl(mt, ct), x_sl(c, ct),
                                     start=(ct == 0), stop=(ct == 1))
                if on_act:   # ACT is idle pre-exp; shortens the startup chain
                    nc.scalar.activation(k2[mt][:, c0:c0 + cw], ps[:, 0:cw],
                                         mybir.ActivationFunctionType.Copy)
                else:
                    nc.vector.tensor_copy(k2[mt][:, c0:c0 + cw], ps[:, 0:cw])

            def emit_qproj(mt, qc):
                q0, qw = QCH[qc]
                ps = apool.tile([128, 512], F32, tag="acc")
                for ct in range(2):
                    nc.tensor.matmul(ps[:, 0:qw], wq_sl(mt, ct), xq_sl(qc, ct),
                                     start=(ct == 0), stop=(ct == 1))
                nc.vector.tensor_scalar_add(
                    qb2[mt][:, q0:q0 + qw], ps[:, 0:qw], bias_sb[:, mt:mt + 1])

            def emit_vproj(t):
                ps = apool.tile([128, 512], F32, tag="acc")
                for ct in range(2):
                    nc.tensor.matmul(ps[:, 0:C], xv_sl(t, ct), wv_sl(ct),
                                     start=(ct == 0), stop=(ct == 1))
                nc.vector.tensor_copy(vT4[:, t, :, 0:HC],
                                      ps[:, 0:C].rearrange("p (h c) -> p h c", c=HC))

            def emit_dots_win(h, w):
                o, nb = WINDOWS[h][w]
                blocks = BLOCKS[h][o:o + nb]
                mt, p = h // 2, h % 2
                rows = slice(p * 64, p * 64 + 64)
                D = dpool.tile([128, WIN], F32, tag="dots")
                for i, (j, t) in enumerate(blocks):
                    nc.tensor.matmul(
                        D[:, i * 128:(i + 1) * 128],
                        k2[mt][rows, t * 128:(t + 1) * 128],
                        qb2[mt][rows, j * 128:(j + 1) * 128],
                        start=True, stop=True,
                        tile_position=(rows.start, 0),
                    )
                nc.scalar.activation(
                    a2[h % 2][:, o * 128:(o + nb) * 128], D[:, 0:nb * 128],
                    mybir.ActivationFunctionType.Exp)

            def emit_attnv(h, j):
                a, pos = a2[h % 2], POS[h]
                P = apool.tile([128, 512], F32, tag="acc")
                for t in range(NT):
                    i = pos[(j, t)]
                    nc.tensor.matmul(
                        P[:, 0:VS],
                        a[:, i * 128:(i + 1) * 128],
                        vT[:, t * VS * HEADS + h * VS: t * VS * HEADS + (h + 1) * VS],
                        start=(t == 0), stop=(t == NT - 1),
                    )
                pr = (3 * h + j) % 2
                nc.vector.reciprocal(rcp[:, pr:pr + 1], P[:, HC:HC + 1])
                nc.vector.tensor_scalar_mul(
                    attT[:, j * 256 + h * 64: j * 256 + (h + 1) * 64],
                    P[:, 0:HC], rcp[:, pr:pr + 1])

            def emit_transpose(j, ct):
                T = apool.tile([128, 512], BF16, tag="acc")
                nc.tensor.transpose(
                    T[:, 0:128], attT[:, j * 256 + ct * 128: j * 256 + (ct + 1) * 128],
                    ident[:, :])
                nc.vector.tensor_copy(att[:, ct * M + j * 128: ct * M + (j + 1) * 128],
                                      T[:, 0:128])

            def emit_unify(j):
                U = apool.tile([128, 512], F32, tag="acc")
                for ot in range(2):
                    for ct in range(2):
                        nc.tensor.matmul(
                            U[:, ot * 128:(ot + 1) * 128],
                            wo_sl(ct, ot),
                            att[:, ct * M + j * 128: ct * M + (j + 1) * 128],
                            start=(ct == 0), stop=(ct == 1))
                for ot in range(2):
                    nc.vector.tensor_scalar_add(
                        y_sb[:, ot * M + j * 128: ot * M + (j + 1) * 128],
                        U[:, ot * 128:(ot + 1) * 128],
                        bias_sb[:, 2 + ot: 3 + ot])
                ysrc = y_sb[:].rearrange("p (o m) -> p o m", o=2)[:, :, j * 128:(j + 1) * 128]
                ydst = y_d.ap()[:].rearrange("(o p) m -> p o m", o=2)[:, :, j * 128:(j + 1) * 128]
                nc.sync.dma_start(ydst, ysrc)

            # ---- schedule ----
            # Per (head, window) slot the PE order is: dots matmuls, exp,
            # then background jobs (projections, attnv, transpose, unify) so
            # the next exp window is always at the head of the PE queue.
            # attnv lags its last exp window by 2 slots (ACT trails PE), and
            # transpose/unify each lag one more slot, so no PE instruction
            # ever stalls on a sem long enough to delay the next dots window.
            post = [dict() for _ in range(HEADS)]
            post[0] = {1: [(emit_kproj, 0, 2)],
                       2: [(emit_qproj, 0, 1), (emit_kproj, 0, 3),
                           (emit_kproj, 0, 4)],
                       4: [(emit_qproj, 0, 2)], 5: [(emit_qproj, 0, 3)]}
            vts = iter(range(NT))
            for w in range(3, 9):
                post[0].setdefault(w, []).extend(
                    (emit_vproj, t) for t in [next(vts) for _ in range(3)])
            for w, js in {10: [0, 1], 11: [2, 3], 12: [4, 5], 13: [6], 14: [7]}.items():
                post[0].setdefault(w, []).extend((emit_attnv, 0, j) for j in js)
            # head-1 projections start in head-0's tail slots to keep PE busy
            # (and at full p-state) across the head transition
            post[0][13] = post[0].get(13, []) + [(emit_kproj, 1, 0)]
            post[0][14] = post[0].get(14, []) + [(emit_kproj, 1, 1)]
            post[1] = {0: [(emit_kproj, 1, 2), (emit_qproj, 1, 0)],
                       1: [(emit_kproj, 1, 3), (emit_qproj, 1, 1)],
                       2: [(emit_kproj, 1, 4), (emit_qproj, 1, 2)],
                       3: [(emit_qproj, 1, 3)]}

            trq, unq = [], []
            for h in range(HEADS):
                for w in range(len(WINDOWS[h])):
                    if h == 0 and w == 0:
                        emit_kproj(0, 0, on_act=True)
                        emit_kproj(0, 1)
                        emit_qproj(0, 0)   # queries of mtiles 0-1
                    emit_dots_win(h, w)
                    for job in post[h].get(w, []):
                        job[0](*job[1:])
                    if h == 2 and 3 <= w < 3 + NJ:
                        emit_transpose(w - 3, 0)
                    if h > 0:
                        if w == 1:
                            emit_attnv(h - 1, NJ - 1)   # previous head's j8
                        for j in sorted(jj for jj, rw in READY[h].items()
                                        if rw == w - 2):
                            emit_attnv(h, j)
                            if h == 3:
                                trq.append((j, w))
                    if h == 3:
                        if trq and trq[0][1] < w:
                            jj = trq.pop(0)[0]
                            emit_transpose(jj, 1)
                            unq.append((jj, w))
                        if unq and unq[0][1] < w:
                            emit_unify(unq.pop(0)[0])
            # tail: drain staged jobs, then the j8 chain
            while trq:
                jj = trq.pop(0)[0]
                emit_transpose(jj, 1)
                unq.append((jj, 99))
            if unq:
                emit_unify(unq.pop(0)[0])
            emit_attnv(3, NJ - 1)
            while unq:
                emit_unify(unq.pop(0)[0])
            emit_transpose(NJ - 1, 1)
            emit_unify(NJ - 1)

    nc.compile()
    return nc


def _get_nc():
    if "nc" not in _CACHED:
        _CACHED["nc"] = build_nc()
    return _CACHED["nc"]


def kernel(x, wk, bk, wq, bq, wv, bv, wo, bo):
    from concourse import bass_utils
    import ml_dtypes

    bf16 = ml_dtypes.bfloat16
    f32 = np.float32
    x = np.asarray(x, dtype=f32)
    wk, wq, wv, wo = (np.asarray(w, dtype=f32) for w in (wk, wq, wv, wo))
    bq, bv, bo = (np.asarray(b_, dtype=f32) for b_ in (bq, bv, bo))

    xb = x.reshape(B, C, N)
    fb = wo @ bv + bo
    bias = np.stack([bq[0:128], bq[128:256], fb[0:128], fb[128:256]],
                    axis=1).astype(f32)

    wkT, wqT, wvT, woT = wk.T, wq.T, wv.T, wo.T
    w3_cols = []
    for ct in range(2):
        rs = slice(ct * 128, (ct + 1) * 128)
        w3_cols += [wkT[rs, 0:128], wkT[rs, 128:256],
                    wqT[rs, 0:128], wqT[rs, 128:256], wvT[rs, :]]
    w3 = np.ascontiguousarray(np.concatenate(w3_cols, axis=1)).astype(bf16)
    woTh = np.ascontiguousarray(
        np.concatenate([woT[0:128, :], woT[128:256, :]], axis=1)).astype(bf16)

    def interleave(arr, chunks):
        blocks = []
        for c0, cw in chunks:
            for ct in range(2):
                blocks.append(arr[ct * 128:(ct + 1) * 128, c0:c0 + cw])
        return np.ascontiguousarray(np.concatenate(blocks, axis=1)).astype(bf16)

    XQCH = [(0, 512), (512, 512), (1024, 128)]
    nc = _get_nc()
    in_maps = []
    xh = {}
    for c in range(8):
        b, qh = c // 2, c % 2
        if b not in xh:
            xh[b] = interleave(xb[b], KCH)
        in_maps.append({
            "x": xh[b],
            "xq": interleave(xb[b][:, qh * M:(qh + 1) * M], XQCH),
            "w3": w3, "woT": woTh, "bias": bias,
        })
    global last_in_maps
    last_in_maps = in_maps
    res = bass_utils.run_bass_kernel_spmd(nc, in_maps, core_ids=list(range(8)))

    out = np.empty((B, C, N), dtype=f32)
    for c in range(8):
        b, qh = c // 2, c % 2
        out[b][:, qh * M:(qh + 1) * M] = res.results[c]["out"]
    return out.reshape(B, C, HW, HW)


# revision 42
# speedup vs baseline: 1.2672x; 1.0085x over previous
"""Trainium2 Bass kernel for nn_Attention (dense_transformer).

Reference (per batch b of 4, dim C=256, HEADS=4, hc=64, N=48*48=2304):
  k = wk@x+bk; q = wq@x+bq; v = wv@x+bv          (1x1 convs = channel GEMMs)
  dots[n,m] = sum_c k[c,n] q[c,m]   per head
  attn = softmax(dots, axis=keys n)
  out  = v @ attn ; y = wo@out + bo

Sharding: 8 cores, core c -> (batch c//2, query-half c%2). Each core computes
all 4 heads for its 1152 queries against all 2304 keys. No collectives.

Design notes (engine-balance driven):
  - exp(dots) can only run on the ACT engine: 82,944 psum columns is the hard
    floor (~80us busy).  Everything else is arranged so ACT streams exp
    windows back-to-back from ~5us on.
  - attn@v runs TRANSPOSED: stationary = exp-weight tile [128 keys, 128 q],
    moving = v^T columns (64 + ones col for sumexp) -> 42k PE cycles instead
    of 83k, giving PE the slack to always stay ahead of ACT.
  - All inputs bf16 (halves DMA, keeps every matmul at 1 cycle/col).
  - bk cancels in softmax over keys; bq folded into the q psum->sbuf copy;
    bv folded via fb = wo@bv + bo (host-precomputed weight constant since
    sum_n attn = 1); sumexp via a ones column appended to v^T.
  - Normalization in the transposed layout is a per-partition scalar multiply
    (queries = partitions), so no gpsimd broadcast is needed.
  - DMAs issue from the otherwise-idle SP ring (HWDGE).  Dummy matmuls at
    t~0 ramp the PE p-state before the first projection.
  - Per-head "block" order (which (mtile, keytile) dots tile goes where in
    the flat exp stream) is a free parameter; head 0 defers the late
    keytiles of mtiles 0-1 so the first windows only need the first x DMAs.
"""

import numpy as np

from concourse import bacc, bass, mybir, tile


F32 = mybir.dt.float32
BF16 = mybir.dt.bfloat16

B, C, HW, HEADS, HC = 4, 256, 48, 4, 64
N = HW * HW          # 2304 keys
M = N // 2           # 1152 queries per core
NT = N // 128        # 18 key tiles
NJ = M // 128        # 9 query tiles (mtiles)
BLK = NT * 128       # 2304 flat cols per mtile
FLAT = NJ * BLK      # 20736 flat dots cols per head
WIN = 1536           # dots psum tile (3 PSUM banks); windows may use less
VS = HC + 1          # v^T cols per head incl. ones col
N_WARMUP = 58        # PE p-state warm-up matmuls (end ~= first x chunk ready)

KCH = [(0, 512), (512, 512), (1024, 512), (1536, 512), (2048, 256)]
QCH = [(0, 256), (256, 256), (512, 512), (1024, 128)]   # j0+j1 split out

# block order per head: list of (j, t) pairs; position in list = position of
# that [128 keys x 128 queries] dots tile in the flat exp stream.  Head 0's
# prefix is ordered (and its first three windows shrunk to 8 blocks) so each
# window only needs x/xq DMA pieces that have already arrived.
_STD = [(j, t) for j in range(NJ) for t in range(NT)]
_H0 = ([(0, t) for t in range(4)] + [(1, t) for t in range(4)]          # w0
       + [(0, t) for t in range(4, 8)] + [(1, t) for t in range(4, 8)]  # w1
       + [(0, t) for t in range(8, 12)] + [(1, t) for t in range(8, 12)]  # w2
       + [(2, t) for t in range(12)]                                    # w3
       + [(0, t) for t in range(12, 18)] + [(1, t) for t in range(12, 18)]  # w4
       + [(2, t) for t in range(12, 18)] + [(3, t) for t in range(6)]   # w5
       + [(3, t) for t in range(6, 18)]                                 # w6
       + [(j, t) for j in range(4, NJ) for t in range(NT)])
BLOCKS = [_H0, _STD, _STD, _STD]
POS = [{jt: i for i, jt in enumerate(bl)} for bl in BLOCKS]
# window sizes (in 128-col blocks) per head; last windows kept >=9 blocks so
# the exp of the final window leaves PE enough budget across head transitions
_WS0 = [8, 8, 8] + [12] * 10 + [9, 9]
_WSS = [12] * 12 + [9, 9]
WINSZ = [_WS0, _WSS, _WSS, _WSS]
# WINDOWS[h] = list of (first_block_index, n_blocks)
WINDOWS = []
for ws in WINSZ:
    offs, o = [], 0
    for n in ws:
        offs.append((o, n))
        o += n
    assert o == NJ * NT
    WINDOWS.append(offs)
# window at which mtile j's last dots tile is produced, per head
READY = []
for h, bl in enumerate(BLOCKS):
    last = {}
    for i, (j, t) in enumerate(bl):
        last[j] = next(w for w, (o, n) in enumerate(WINDOWS[h]) if o <= i < o + n)
    READY.append(last)

_CACHED = {}
last_in_maps = None


def build_nc():
    nc = bacc.Bacc("TRN2", target_bir_lowering=False, debug=False)

    x_d = nc.dram_tensor("x", [128, 2 * N], BF16, kind="ExternalInput")
    xq_d = nc.dram_tensor("xq", [128, 2 * M], BF16, kind="ExternalInput")
    w3_d = nc.dram_tensor("w3", [128, 6 * C], BF16, kind="ExternalInput")
    woT_d = nc.dram_tensor("woT", [128, 2 * C], BF16, kind="ExternalInput")
    bias_d = nc.dram_tensor("bias", [128, 4], F32, kind="ExternalInput")
    y_d = nc.dram_tensor("out", [C, M], F32, kind="ExternalOutput")

    sb = lambda name, shape, dt: nc.alloc_sbuf_tensor(name, shape, dt).ap()

    x_sb = sb("x_sb", [128, 2 * N], BF16)
    xq_sb = sb("xq_sb", [128, 2 * M], BF16)
    w3_sb = sb("w3_sb", [128, 6 * C], BF16)
    woT_sb = sb("woT_sb", [128, 2 * C], BF16)
    bias_sb = sb("bias_sb", [128, 4], F32)
    k2 = [sb(f"k2_{mt}", [128, N], BF16) for mt in range(2)]
    qb2 = [sb(f"qb2_{mt}", [128, M], BF16) for mt in range(2)]
    vT = sb("vT", [128, NT * VS * HEADS], BF16)
    a2 = [sb(f"a{i}", [128, FLAT], BF16) for i in range(2)]
    attT = sb("attT", [128, NJ * 256], BF16)
    att = sb("att", [128, 2 * M], BF16)
    rcp = sb("rcp", [128, 2], F32)
    y_sb = sb("y_sb", [128, 2 * M], F32)
    ident = sb("ident", [128, 128], BF16)
    scratch = sb("scratch", [128, 128], BF16)
    warm = sb("warm", [1, 2], F32)

    # sbuf slice helpers
    def x_sl(c, ct):
        c0, cw = KCH[c]
        base = 1024 * c if c < 4 else 4096
        return x_sb[:, base + ct * cw: base + (ct + 1) * cw]

    def xv_sl(t, ct):
        c = t // 4
        cw = KCH[c][1]
        base = 1024 * c if c < 4 else 4096
        o = base + ct * cw + 128 * (t % 4)
        return x_sb[:, o: o + 128]

    def xq_sl(qc, ct):
        q0, qw = QCH[qc]
        pbase, pq0, pcw = (0, 0, 512) if q0 < 512 else \
                          (1024, 512, 512) if q0 < 1024 else (2048, 1024, 128)
        o = pbase + ct * pcw + (q0 - pq0)
        return xq_sb[:, o: o + qw]

    def xq_dma(q0, qw):      # strided piece covering both ct halves
        v = xq_d.ap()[:, 0:1024].rearrange("p (ct m) -> p ct m", ct=2)
        s = xq_sb[:, 0:1024].rearrange("p (ct m) -> p ct m", ct=2)
        nc.sync.dma_start(s[:, :, q0:q0 + qw], v[:, :, q0:q0 + qw])

    wk_sl = lambda mt, ct: w3_sb[:, ct * 768 + mt * 128: ct * 768 + (mt + 1) * 128]
    wq_sl = lambda mt, ct: w3_sb[:, ct * 768 + 256 + mt * 128: ct * 768 + 256 + (mt + 1) * 128]
    wv_sl = lambda ct: w3_sb[:, ct * 768 + 512: ct * 768 + 768]
    wo_sl = lambda ct, ot: woT_sb[:, ct * 256 + ot * 128: ct * 256 + (ot + 1) * 128]

    with tile.TileContext(nc) as tc, nc.allow_low_precision(
            reason="attention weights; rel-err budget 2e-2"):
        with (
            tc.tile_pool(name="ps_dots", bufs=2, space="PSUM") as dpool,
            tc.tile_pool(name="ps_acc", bufs=2, space="PSUM") as apool,
        ):
            # ---- input DMAs on the idle SP ring, critical pieces first ----
            # (a consumer sees DMA data ~900ns after transfer end: sem prop)
            w3v = w3_d.ap()[:].rearrange("p (ct q) -> p ct q", ct=2)
            w3s = w3_sb[:].rearrange("p (ct q) -> p ct q", ct=2)
            dma = nc.sync.dma_start
            dma(w3s[:, :, 0:128], w3v[:, :, 0:128])          # wk mt0
            dma(x_sb[:, 0:1024], x_d.ap()[:, 0:1024])        # x chunk 0
            dma(x_sb[:, 1024:2048], x_d.ap()[:, 1024:2048])  # x chunk 1
            dma(w3s[:, :, 256:384], w3v[:, :, 256:384])      # wq mt0
            xq_dma(0, 256)                                   # q cols of j0+j1
            dma(bias_sb[:, :], bias_d.ap()[:, :])
            dma(x_sb[:, 2048:3072], x_d.ap()[:, 2048:3072])  # x chunk 2
            dma(x_sb[:, 3072:4096], x_d.ap()[:, 3072:4096])  # x chunk 3
            xq_dma(256, 256)                                 # q cols j2+j3
            dma(x_sb[:, 4096:4608], x_d.ap()[:, 4096:4608])  # x chunk 4
            dma(w3s[:, :, 128:256], w3v[:, :, 128:256])      # wk mt1
            dma(w3s[:, :, 384:768], w3v[:, :, 384:768])      # wq mt1 + wv
            dma(xq_sb[:, 1024:2304], xq_d.ap()[:, 1024:2304])
            dma(woT_sb[:, :], woT_d.ap()[:, :])

            # warm the ACT exp table while DMAs/warm-up matmuls run
            nc.vector.memset(warm[:, :], 0.0)
            nc.scalar.activation(warm[:, :], warm[:, :],
                                 mybir.ActivationFunctionType.Exp)

            # PE p-state warm-up (full speed needs ~3us of continuous PE work)
            nc.vector.memset(scratch[:, :], 0.0)
            dtile = apool.tile([128, 512], F32, tag="acc")
            for _ in range(N_WARMUP):
                nc.tensor.matmul(dtile[:, 0:64], scratch[:, 0:128],
                                 scratch[:, 0:64], start=True, stop=True)

            # gpsimd setup (Pool is otherwise idle)
            from concourse.masks import make_identity
            vT4 = vT[:].rearrange("p (t h s) -> p t h s", t=NT, h=HEADS)
            nc.gpsimd.memset(vT4[:, :, :, HC:VS], 1.0)
            make_identity(nc, ident[:, :])

            # ---- emit helpers ----
            def emit_kproj(mt, c, on_act=False):
                c0, cw = KCH[c]
                ps = apool.tile([128, 512], F32, tag="acc")
                for ct in range(2):
                    nc.tensor.matmul(ps[:, 0:cw], wk_sl(mt, ct), x_sl(c, ct),
                                     start=(ct == 0), stop=(ct == 1))
                if on_act:   # ACT is idle pre-exp; shortens the startup chain
                    nc.scalar.activation(k2[mt][:, c0:c0 + cw], ps[:, 0:cw],
                                         mybir.ActivationFunctionType.Copy)
                else:
                    nc.vector.tensor_copy(k2[mt][:, c0:c0 + cw], ps[:, 0:cw])

            def emit_qproj(mt, qc):
                q0, qw = QCH[qc]
                ps = apool.tile([128, 512], F32, tag="acc")
                for ct in range(2):
                    nc.tensor.matmul(ps[:, 0:qw], wq_sl(mt, ct), xq_sl(qc, ct),
                                     start=(ct == 0), stop=(ct == 1))
                nc.vector.tensor_scalar_add(
                    qb2[mt][:, q0:q0 + qw], ps[:, 0:qw], bias_sb[:, mt:mt + 1])

            def emit_vproj(t):
                ps = apool.tile([128, 512], F32, tag="acc")
                for ct in range(2):
                    nc.tensor.matmul(ps[:, 0:C], xv_sl(t, ct), wv_sl(ct),
                                     start=(ct == 0), stop=(ct == 1))
                nc.vector.tensor_copy(vT4[:, t, :, 0:HC],
                                      ps[:, 0:C].rearrange("p (h c) -> p h c", c=HC))

            def emit_dots_win(h, w):
                o, nb = WINDOWS[h][w]
                blocks = BLOCKS[h][o:o + nb]
                mt, p = h // 2, h % 2
                rows = slice(p * 64, p * 64 + 64)
                D = dpool.tile([128, WIN], F32, tag="dots")
                for i, (j, t) in enumerate(blocks):
                    nc.tensor.matmul(
                        D[:, i * 128:(i + 1) * 128],
                        k2[mt][rows, t * 128:(t + 1) * 128],
                        qb2[mt][rows, j * 128:(j + 1) * 128],
                        start=True, stop=True,
                        tile_position=(rows.start, 0),
                    )
                nc.scalar.activation(
                    a2[h % 2][:, o * 128:(o + nb) * 128], D[:, 0:nb * 128],
                    mybir.ActivationFunctionType.Exp)

            def emit_attnv(h, j):
                a, pos = a2[h % 2], POS[h]
                P = apool.tile([128, 512], F32, tag="acc")
                for t in range(NT):
                    i = pos[(j, t)]
                    nc.tensor.matmul(
                        P[:, 0:VS],
                        a[:, i * 128:(i + 1) * 128],
                        vT[:, t * VS * HEADS + h * VS: t * VS * HEADS + (h + 1) * VS],
                        start=(t == 0), stop=(t == NT - 1),
                    )
                pr = (3 * h + j) % 2
                nc.vector.reciprocal(rcp[:, pr:pr + 1], P[:, HC:HC + 1])
                nc.vector.tensor_scalar_mul(
                    attT[:, j * 256 + h * 64: j * 256 + (h + 1) * 64],
                    P[:, 0:HC], rcp[:, pr:pr + 1])

            def emit_transpose(j, ct):
                T = apool.tile([128, 512], BF16, tag="acc")
                nc.tensor.transpose(
                    T[:, 0:128], attT[:, j * 256 + ct * 128: j * 256 + (ct + 1) * 128],
                    ident[:, :])
                nc.vector.tensor_copy(att[:, ct * M + j * 128: ct * M + (j + 1) * 128],
                                      T[:, 0:128])

            def emit_unify(j):
                U = apool.tile([128, 512], F32, tag="acc")
                for ot in range(2):
                    for ct in range(2):
                        nc.tensor.matmul(
                            U[:, ot * 128:(ot + 1) * 128],
                            wo_sl(ct, ot),
                            att[:, ct * M + j * 128: ct * M + (j + 1) * 128],
                            start=(ct == 0), stop=(ct == 1))
                for ot in range(2):
                    nc.vector.tensor_scalar_add(
                        y_sb[:, ot * M + j * 128: ot * M + (j + 1) * 128],
                        U[:, ot * 128:(ot + 1) * 128],
                        bias_sb[:, 2 + ot: 3 + ot])
                ysrc = y_sb[:].rearrange("p (o m) -> p o m", o=2)[:, :, j * 128:(j + 1) * 128]
                ydst = y_d.ap()[:].rearrange("(o p) m -> p o m", o=2)[:, :, j * 128:(j + 1) * 128]
                nc.sync.dma_start(ydst, ysrc)

            # ---- schedule ----
            # Per (head, window) slot the PE order is: dots matmuls, exp,
            # then background jobs (projections, attnv, transpose, unify) so
            # the next exp window is always at the head of the PE queue.
            # attnv lags its last exp window by 2 slots (ACT trails PE), and
            # transpose/unify each lag one more slot, so no PE instruction
            # ever stalls on a sem long enough to delay the next dots window.
            post = [dict() for _ in range(HEADS)]
            post[0] = {1: [(emit_kproj, 0, 2)],
                       2: [(emit_qproj, 0, 1), (emit_kproj, 0, 3)],
                       3: [(emit_kproj, 0, 4)],
                       4: [(emit_qproj, 0, 2)], 5: [(emit_qproj, 0, 3)]}
            vts = iter(range(NT))
            for w in range(4, 10):
                post[0].setdefault(w, []).extend(
                    (emit_vproj, t) for t in [next(vts) for _ in range(3)])
            for w, js in {10: [0], 11: [1], 12: [2], 13: [3], 14: [4]}.items():
                post[0].setdefault(w, []).extend((emit_attnv, 0, j) for j in js)
            # head 0's overflow attnvs and head-1/2 projections are spread one
            # job per slot across head 1's free windows (its own attnv stream
            # occupies slots 3,4,6,7,9,10,12,13) so no slot exceeds the
            # per-window ACT budget and PE keeps its p-state at transitions.
            post[1] = {0: [(emit_attnv, 0, 5)], 1: [(emit_attnv, 0, 6)],
                       2: [(emit_attnv, 0, 7)], 5: [(emit_attnv, 0, 8)],
                       3: [(emit_qproj, 1, 0)], 4: [(emit_qproj, 1, 1)],
                       6: [(emit_qproj, 1, 2)], 7: [(emit_qproj, 1, 3)],
                       8: [(emit_kproj, 1, 0), (emit_kproj, 1, 1)],
                       11: [(emit_kproj, 1, 2), (emit_kproj, 1, 3)]}
            post[2] = {0: [(emit_kproj, 1, 4)]}

            trq, unq = [], []
            for h in range(HEADS):
                for w in range(len(WINDOWS[h])):
                    if h == 0 and w == 0:
                        emit_kproj(0, 0, on_act=True)
                        emit_kproj(0, 1, on_act=True)
                        emit_qproj(0, 0)   # queries of mtiles 0-1
                    emit_dots_win(h, w)
                    for job in post[h].get(w, []):
                        job[0](*job[1:])
                    if h == 2 and 3 <= w < 3 + NJ:
                        emit_transpose(w - 3, 0)
                    if h > 0:
                        if w == 1 and h > 1:
                            emit_attnv(h - 1, NJ - 1)   # previous head's j8
                        for j in sorted(jj for jj, rw in READY[h].items()
                                        if rw == w - 2):
                            emit_attnv(h, j)
                            if h == 3:
                                trq.append((j, w))
                    if h == 3:
                        if trq and trq[0][1] < w:
                            jj = trq.pop(0)[0]
                            emit_transpose(jj, 1)
                            unq.append((jj, w))
                        if unq and unq[0][1] < w:
                            emit_unify(unq.pop(0)[0])
            # tail: drain staged jobs, then the j8 chain
            while trq:
                jj = trq.pop(0)[0]
                emit_transpose(jj, 1)
                unq.append((jj, 99))
            if unq:
                emit_unify(unq.pop(0)[0])
            emit_attnv(3, NJ - 1)
            while unq:
                emit_unify(unq.pop(0)[0])
            emit_transpose(NJ - 1, 1)
            emit_unify(NJ - 1)

    nc.compile()
    return nc


def _get_nc():
    if "nc" not in _CACHED:
        _CACHED["nc"] = build_nc()
    return _CACHED["nc"]


def kernel(x, wk, bk, wq, bq, wv, bv, wo, bo):
    from concourse import bass_utils
    import ml_dtypes

    bf16 = ml_dtypes.bfloat16
    f32 = np.float32
    x = np.asarray(x, dtype=f32)
    wk, wq, wv, wo = (np.asarray(w, dtype=f32) for w in (wk, wq, wv, wo))
    bq, bv, bo = (np.asarray(b_, dtype=f32) for b_ in (bq, bv, bo))

    xb = x.reshape(B, C, N)
    fb = wo @ bv + bo
    bias = np.stack([bq[0:128], bq[128:256], fb[0:128], fb[128:256]],
                    axis=1).astype(f32)

    wkT, wqT, wvT, woT = wk.T, wq.T, wv.T, wo.T
    w3_cols = []
    for ct in range(2):
        rs = slice(ct * 128, (ct + 1) * 128)
        w3_cols += [wkT[rs, 0:128], wkT[rs, 128:256],
                    wqT[rs, 0:128], wqT[rs, 128:256], wvT[rs, :]]
    w3 = np.ascontiguousarray(np.concatenate(w3_cols, axis=1)).astype(bf16)
    woTh = np.ascontiguousarray(
        np.concatenate([woT[0:128, :], woT[128:256, :]], axis=1)).astype(bf16)

    def interleave(arr, chunks):
        blocks = []
        for c0, cw in chunks:
            for ct in range(2):
                blocks.append(arr[ct * 128:(ct + 1) * 128, c0:c0 + cw])
        return np.ascontiguousarray(np.concatenate(blocks, axis=1)).astype(bf16)

    XQCH = [(0, 512), (512, 512), (1024, 128)]
    nc = _get_nc()
    in_maps = []
    xh = {}
    for c in range(8):
        b, qh = c // 2, c % 2
        if b not in xh:
            xh[b] = interleave(xb[b], KCH)
        in_maps.append({
            "x": xh[b],
            "xq": interleave(xb[b][:, qh * M:(qh + 1) * M], XQCH),
            "w3": w3, "woT": woTh, "bias": bias,
        })
    global last_in_maps
    last_in_maps = in_maps
    res = bass_utils.run_bass_kernel_spmd(nc, in_maps, core_ids=list(range(8)))

    out = np.empty((B, C, N), dtype=f32)
    for c in range(8):
        b, qh = c // 2, c % 2
        out[b][:, qh * M:(qh + 1) * M] = res.results[c]["out"]
    return out.reshape(B, C, HW, HW)


# revision 86
# speedup vs baseline: 1.2856x; 1.0145x over previous
"""Trainium2 Bass kernel for nn_Attention (dense_transformer).

Reference (per batch b of 4, dim C=256, HEADS=4, hc=64, N=48*48=2304):
  k = wk@x+bk; q = wq@x+bq; v = wv@x+bv          (1x1 convs = channel GEMMs)
  dots[n,m] = sum_c k[c,n] q[c,m]   per head
  attn = softmax(dots, axis=keys n)
  out  = v @ attn ; y = wo@out + bo

Sharding: 8 cores, core c -> (batch c//2, query-half c%2). Each core computes
all 4 heads for its 1152 queries against all 2304 keys. No collectives.

Design notes (engine-balance driven):
  - exp(dots) can only run on the ACT engine: 82,944 psum columns is the hard
    floor (~80us busy).  Everything else is arranged so ACT streams exp
    windows back-to-back from ~5us on.
  - attn@v runs TRANSPOSED: stationary = exp-weight tile [128 keys, 128 q],
    moving = v^T columns (64 + ones col for sumexp) -> 42k PE cycles instead
    of 83k, giving PE the slack to always stay ahead of ACT.
  - All inputs bf16 (halves DMA, keeps every matmul at 1 cycle/col).
  - bk cancels in softmax over keys; bq folded into the q psum->sbuf copy;
    bv folded via fb = wo@bv + bo (host-precomputed weight constant since
    sum_n attn = 1); sumexp via a ones column appended to v^T.
  - Normalization in the transposed layout is a per-partition scalar multiply
    (queries = partitions), so no gpsimd broadcast is needed.
  - DMAs issue from the otherwise-idle SP ring (HWDGE).  Dummy matmuls at
    t~0 ramp the PE p-state before the first projection.
  - Per-head "block" order (which (mtile, keytile) dots tile goes where in
    the flat exp stream) is a free parameter; head 0 defers the late
    keytiles of mtiles 0-1 so the first windows only need the first x DMAs.
"""

import numpy as np

from concourse import bacc, bass, mybir, tile


F32 = mybir.dt.float32
BF16 = mybir.dt.bfloat16

B, C, HW, HEADS, HC = 4, 256, 48, 4, 64
N = HW * HW          # 2304 keys
M = N // 2           # 1152 queries per core
NT = N // 128        # 18 key tiles
NJ = M // 128        # 9 query tiles (mtiles)
BLK = NT * 128       # 2304 flat cols per mtile
FLAT = NJ * BLK      # 20736 flat dots cols per head
WIN = 1536           # dots psum tile (3 PSUM banks); windows may use less
VS = HC + 1          # v^T cols per head incl. ones col
N_WARMUP = 54        # PE p-state warm-up matmuls (end ~= first x chunk ready)

KCH = [(0, 512), (512, 512), (1024, 512), (1536, 512), (2048, 256)]
QCH = [(0, 256), (256, 256), (512, 256), (768, 256), (1024, 128)]

# block order per head: list of (j, t) pairs; position in list = position of
# that [128 keys x 128 queries] dots tile in the flat exp stream.  Head 0's
# prefix is ordered (and its first three windows shrunk to 8 blocks) so each
# window only needs x/xq DMA pieces that have already arrived.
_STD = [(j, t) for j in range(NJ) for t in range(NT)]
_H0 = ([(0, t) for t in range(4)] + [(1, t) for t in range(4)]          # w0
       + [(0, t) for t in range(4, 8)] + [(1, t) for t in range(4, 8)]  # w1
       + [(0, t) for t in range(8, 12)] + [(1, t) for t in range(8, 12)]  # w2
       + [(2, t) for t in range(12)]                                    # w3
       + [(0, t) for t in range(12, 18)] + [(1, t) for t in range(12, 18)]  # w4
       + [(2, t) for t in range(12, 18)] + [(3, t) for t in range(6)]   # w5
       + [(3, t) for t in range(6, 18)]                                 # w6
       + [(j, t) for j in range(4, NJ) for t in range(NT)])
BLOCKS = [_H0, _STD, _STD, _STD]
POS = [{jt: i for i, jt in enumerate(bl)} for bl in BLOCKS]
# window sizes (in 128-col blocks) per head; last windows kept >=9 blocks so
# the exp of the final window leaves PE enough budget across head transitions
_WS0 = [8, 8, 8] + [12] * 10 + [9, 9]
_WSS = [12] * 12 + [9, 9]
WINSZ = [_WS0, _WSS, _WSS, _WSS]
# WINDOWS[h] = list of (first_block_index, n_blocks)
WINDOWS = []
for ws in WINSZ:
    offs, o = [], 0
    for n in ws:
        offs.append((o, n))
        o += n
    assert o == NJ * NT
    WINDOWS.append(offs)
# window at which mtile j's last dots tile is produced, per head
READY = []
for h, bl in enumerate(BLOCKS):
    last = {}
    for i, (j, t) in enumerate(bl):
        last[j] = next(w for w, (o, n) in enumerate(WINDOWS[h]) if o <= i < o + n)
    READY.append(last)

_CACHED = {}
last_in_maps = None


def build_nc():
    nc = bacc.Bacc("TRN2", target_bir_lowering=False, debug=False)

    x_d = nc.dram_tensor("x", [128, 2 * N], BF16, kind="ExternalInput")
    xq_d = nc.dram_tensor("xq", [128, 2 * M], BF16, kind="ExternalInput")
    w3_d = nc.dram_tensor("w3", [128, 6 * C], BF16, kind="ExternalInput")
    woT_d = nc.dram_tensor("woT", [128, 2 * C], BF16, kind="ExternalInput")
    bias_d = nc.dram_tensor("bias", [128, 4], F32, kind="ExternalInput")
    y_d = nc.dram_tensor("out", [C, M], F32, kind="ExternalOutput")

    sb = lambda name, shape, dt: nc.alloc_sbuf_tensor(name, shape, dt).ap()

    x_sb = sb("x_sb", [128, 2 * N], BF16)
    xq_sb = sb("xq_sb", [128, 2 * M], BF16)
    w3_sb = sb("w3_sb", [128, 6 * C], BF16)
    woT_sb = sb("woT_sb", [128, 2 * C], BF16)
    bias_sb = sb("bias_sb", [128, 4], F32)
    k2 = [sb(f"k2_{mt}", [128, N], BF16) for mt in range(2)]
    qb2 = [sb(f"qb2_{mt}", [128, M], BF16) for mt in range(2)]
    vT = sb("vT", [128, NT * VS * HEADS], BF16)
    a2 = [sb(f"a{i}", [128, FLAT], BF16) for i in range(2)]
    attT = sb("attT", [128, NJ * 256], BF16)
    att = sb("att", [128, 2 * M], BF16)
    rcp = sb("rcp", [128, 2], F32)
    y_sb = sb("y_sb", [128, 2 * M], F32)
    ident = sb("ident", [128, 128], BF16)
    scratch = sb("scratch", [128, 128], BF16)
    warm = sb("warm", [1, 2], F32)

    # sbuf slice helpers
    def x_sl(c, ct, off=0, w=None):
        c0, cw = KCH[c]
        base = 1024 * c if c < 4 else 4096
        o = base + ct * cw + off
        return x_sb[:, o: o + (w or cw)]

    def xv_sl(t, ct):
        c = t // 4
        cw = KCH[c][1]
        base = 1024 * c if c < 4 else 4096
        o = base + ct * cw + 128 * (t % 4)
        return x_sb[:, o: o + 128]

    def xq_sl(qc, ct):
        q0, qw = QCH[qc]
        pbase, pq0, pcw = (0, 0, 512) if q0 < 512 else \
                          (1024, 512, 512) if q0 < 1024 else (2048, 1024, 128)
        o = pbase + ct * pcw + (q0 - pq0)
        return xq_sb[:, o: o + qw]

    def xq_dma(q0, qw):      # strided piece covering both ct halves
        v = xq_d.ap()[:, 0:1024].rearrange("p (ct m) -> p ct m", ct=2)
        s = xq_sb[:, 0:1024].rearrange("p (ct m) -> p ct m", ct=2)
        nc.sync.dma_start(s[:, :, q0:q0 + qw], v[:, :, q0:q0 + qw])

    wk_sl = lambda mt, ct: w3_sb[:, ct * 768 + mt * 128: ct * 768 + (mt + 1) * 128]
    wq_sl = lambda mt, ct: w3_sb[:, ct * 768 + 256 + mt * 128: ct * 768 + 256 + (mt + 1) * 128]
    wv_sl = lambda ct: w3_sb[:, ct * 768 + 512: ct * 768 + 768]
    wo_sl = lambda ct, ot: woT_sb[:, ct * 256 + ot * 128: ct * 256 + (ot + 1) * 128]

    with tile.TileContext(nc) as tc, nc.allow_low_precision(
            reason="attention weights; rel-err budget 2e-2"):
        with (
            tc.tile_pool(name="ps_dots", bufs=2, space="PSUM") as dpool,
            tc.tile_pool(name="ps_acc", bufs=2, space="PSUM") as apool,
        ):
            # ---- input DMAs: the most critical pieces (x0, x1, q cols of
            # j0/j1) go through the software-DGE path on the idle Pool ring,
            # whose descriptor generation runs in PARALLEL with the SP ring's
            # hardware-DGE queue; everything else streams on the SP ring.
            # (a consumer sees DMA data ~900ns after transfer end: sem prop)
            w3v = w3_d.ap()[:].rearrange("p (ct q) -> p ct q", ct=2)
            w3s = w3_sb[:].rearrange("p (ct q) -> p ct q", ct=2)
            xqv = xq_d.ap()[:, 0:1024].rearrange("p (ct m) -> p ct m", ct=2)
            xqs = xq_sb[:, 0:1024].rearrange("p (ct m) -> p ct m", ct=2)
            nc.gpsimd.dma_start(x_sb[:, 0:1024], x_d.ap()[:, 0:1024])      # x0
            dma = nc.sync.dma_start
            dma(w3s[:, :, 0:128], w3v[:, :, 0:128])          # wk mt0
            dma(x_sb[:, 1024:2048], x_d.ap()[:, 1024:2048])  # x chunk 1
            dma(w3s[:, :, 256:384], w3v[:, :, 256:384])      # wq mt0
            dma(xqs[:, :, 0:256], xqv[:, :, 0:256])          # q cols of j0+j1
            dma(bias_sb[:, :], bias_d.ap()[:, :])
            dma(x_sb[:, 2048:3072], x_d.ap()[:, 2048:3072])  # x chunk 2
            dma(x_sb[:, 3072:4096], x_d.ap()[:, 3072:4096])  # x chunk 3
            xq_dma(256, 256)                                 # q cols j2+j3
            dma(x_sb[:, 4096:4608], x_d.ap()[:, 4096:4608])  # x chunk 4
            dma(w3s[:, :, 128:256], w3v[:, :, 128:256])      # wk mt1
            dma(w3s[:, :, 384:768], w3v[:, :, 384:768])      # wq mt1 + wv
            dma(xq_sb[:, 1024:2304], xq_d.ap()[:, 1024:2304])
            dma(woT_sb[:, :], woT_d.ap()[:, :])

            # warm the ACT exp table while DMAs/warm-up matmuls run
            nc.vector.memset(warm[:, :], 0.0)
            nc.scalar.activation(warm[:, :], warm[:, :],
                                 mybir.ActivationFunctionType.Exp)

            # PE p-state warm-up (full speed needs ~3us of continuous PE work)
            nc.vector.memset(scratch[:, :], 0.0)
            dtile = apool.tile([128, 512], F32, tag="acc")
            for _ in range(N_WARMUP):
                nc.tensor.matmul(dtile[:, 0:64], scratch[:, 0:128],
                                 scratch[:, 0:64], start=True, stop=True)

            # gpsimd setup (Pool is otherwise idle)
            from concourse.masks import make_identity
            vT4 = vT[:].rearrange("p (t h s) -> p t h s", t=NT, h=HEADS)
            nc.gpsimd.memset(vT4[:, :, :, HC:VS], 1.0)
            make_identity(nc, ident[:, :])

            # ---- emit helpers ----
            def emit_kproj(mt, c, on_act=False, half=None):
                c0, cw = KCH[c]
                off = 0
                if half is not None:     # 256-col sub-chunk for finer overlap
                    off, cw = half * 256, 256
                ps = apool.tile([128, 512], F32, tag="acc")
                for ct in range(2):
                    nc.tensor.matmul(ps[:, 0:cw],
                                     wk_sl(mt, ct), x_sl(c, ct, off, cw),
                                     start=(ct == 0), stop=(ct == 1))
                if on_act:   # ACT is idle pre-exp; shortens the startup chain
                    nc.scalar.activation(k2[mt][:, c0 + off:c0 + off + cw],
                                         ps[:, 0:cw],
                                         mybir.ActivationFunctionType.Copy)
                else:
                    nc.vector.tensor_copy(k2[mt][:, c0 + off:c0 + off + cw],
                                          ps[:, 0:cw])

            def emit_qproj(mt, qc):
                q0, qw = QCH[qc]
                ps = apool.tile([128, 512], F32, tag="acc")
                for ct in range(2):
                    nc.tensor.matmul(ps[:, 0:qw], wq_sl(mt, ct), xq_sl(qc, ct),
                                     start=(ct == 0), stop=(ct == 1))
                nc.vector.tensor_scalar_add(
                    qb2[mt][:, q0:q0 + qw], ps[:, 0:qw], bias_sb[:, mt:mt + 1])

            def emit_vproj(t):       # keytile pair t, t+1: one copy op
                ps = apool.tile([128, 512], F32, tag="acc")
                for i in range(2):
                    for ct in range(2):
                        nc.tensor.matmul(ps[:, i * C:i * C + C],
                                         xv_sl(t + i, ct), wv_sl(ct),
                                         start=(ct == 0), stop=(ct == 1))
                nc.vector.tensor_copy(
                    vT4[:, t:t + 2, :, 0:HC],
                    ps[:, 0:2 * C].rearrange("p (t h c) -> p t h c", t=2, c=HC))

            def emit_dots_win(h, w):
                o, nb = WINDOWS[h][w]
                blocks = BLOCKS[h][o:o + nb]
                mt, p = h // 2, h % 2
                rows = slice(p * 64, p * 64 + 64)
                D = dpool.tile([128, WIN], F32, tag="dots")
                for i, (j, t) in enumerate(blocks):
                    nc.tensor.matmul(
                        D[:, i * 128:(i + 1) * 128],
                        k2[mt][rows, t * 128:(t + 1) * 128],
                        qb2[mt][rows, j * 128:(j + 1) * 128],
                        start=True, stop=True,
                        tile_position=(rows.start, 0),
                    )
                nc.scalar.activation(
                    a2[h % 2][:, o * 128:(o + nb) * 128], D[:, 0:nb * 128],
                    mybir.ActivationFunctionType.Exp)

            def emit_attnv(h, j):
                a, pos = a2[h % 2], POS[h]
                P = apool.tile([128, 512], F32, tag="acc")
                for t in range(NT):
                    i = pos[(j, t)]
                    nc.tensor.matmul(
                        P[:, 0:VS],
                        a[:, i * 128:(i + 1) * 128],
                        vT[:, t * VS * HEADS + h * VS: t * VS * HEADS + (h + 1) * VS],
                        start=(t == 0), stop=(t == NT - 1),
                    )
                pr = (3 * h + j) % 2
                nc.vector.reciprocal(rcp[:, pr:pr + 1], P[:, HC:HC + 1])
                nc.vector.tensor_scalar_mul(
                    attT[:, j * 256 + h * 64: j * 256 + (h + 1) * 64],
                    P[:, 0:HC], rcp[:, pr:pr + 1])

            def emit_transpose(j, ct):
                T = apool.tile([128, 512], BF16, tag="acc")
                nc.tensor.transpose(
                    T[:, 0:128], attT[:, j * 256 + ct * 128: j * 256 + (ct + 1) * 128],
                    ident[:, :])
                nc.vector.tensor_copy(att[:, ct * M + j * 128: ct * M + (j + 1) * 128],
                                      T[:, 0:128])

            def emit_unify(j):
                U = apool.tile([128, 512], F32, tag="acc")
                for ot in range(2):
                    for ct in range(2):
                        nc.tensor.matmul(
                            U[:, ot * 128:(ot + 1) * 128],
                            wo_sl(ct, ot),
                            att[:, ct * M + j * 128: ct * M + (j + 1) * 128],
                            start=(ct == 0), stop=(ct == 1))
                for ot in range(2):
                    nc.vector.tensor_scalar_add(
                        y_sb[:, ot * M + j * 128: ot * M + (j + 1) * 128],
                        U[:, ot * 128:(ot + 1) * 128],
                        bias_sb[:, 2 + ot: 3 + ot])
                ysrc = y_sb[:].rearrange("p (o m) -> p o m", o=2)[:, :, j * 128:(j + 1) * 128]
                ydst = y_d.ap()[:].rearrange("(o p) m -> p o m", o=2)[:, :, j * 128:(j + 1) * 128]
                nc.sync.dma_start(ydst, ysrc)

            # ---- schedule ----
            # Per (head, window) slot the PE order is: dots matmuls, exp,
            # then background jobs (projections, attnv, transpose, unify) so
            # the next exp window is always at the head of the PE queue.
            # attnv lags its last exp window by 2 slots (ACT trails PE), and
            # transpose/unify each lag one more slot, so no PE instruction
            # ever stalls on a sem long enough to delay the next dots window.
            post = [dict() for _ in range(HEADS)]
            post[0] = {1: [(emit_kproj, 0, 2)],
                       2: [(emit_qproj, 0, 1), (emit_kproj, 0, 3, False, 0),
                           (emit_kproj, 0, 3, False, 1)],
                       3: [(emit_kproj, 0, 4)],
                       6: [(emit_qproj, 0, 2)], 7: [(emit_qproj, 0, 3)],
                       8: [(emit_qproj, 0, 4)]}
            vts = iter(range(0, NT, 2))
            for w, nv in {3: 1, 4: 2, 5: 2, 6: 1, 7: 1, 8: 1, 9: 1}.items():
                post[0].setdefault(w, []).extend(
                    (emit_vproj, t) for t in [next(vts) for _ in range(nv)])
            for w, js in {10: [0], 11: [1], 12: [2], 13: [3], 14: [4]}.items():
                post[0].setdefault(w, []).extend((emit_attnv, 0, j) for j in js)
            # head 0's overflow attnvs and head-1/2 projections are spread one
            # job per slot across head 1's free windows (its own attnv stream
            # occupies slots 3,4,6,7,9,10,12,13) so no slot exceeds the
            # per-window ACT budget and PE keeps its p-state at transitions.
            post[1] = {0: [(emit_attnv, 0, 5)], 1: [(emit_attnv, 0, 6)],
                       2: [(emit_attnv, 0, 7)], 5: [(emit_attnv, 0, 8)],
                       3: [(emit_qproj, 1, 0)], 4: [(emit_qproj, 1, 1)],
                       6: [(emit_qproj, 1, 2)], 7: [(emit_qproj, 1, 3)],
                       9: [(emit_qproj, 1, 4)],
                       8: [(emit_kproj, 1, 0), (emit_kproj, 1, 1)],
                       11: [(emit_kproj, 1, 2), (emit_kproj, 1, 3)]}
            post[2] = {0: [(emit_kproj, 1, 4)]}

            trq, unq = [], []
            for h in range(HEADS):
                for w in range(len(WINDOWS[h])):
                    if h == 0 and w == 0:
                        emit_kproj(0, 0, on_act=True)
                        emit_kproj(0, 1, on_act=True)
                        emit_qproj(0, 0)   # queries of mtiles 0-1
                    emit_dots_win(h, w)
                    for job in post[h].get(w, []):
                        job[0](*job[1:])
                    if h == 2 and 3 <= w < 3 + NJ:
                        emit_transpose(w - 3, 0)
                    if h > 0:
                        if w == 1 and h > 1:
                            emit_attnv(h - 1, NJ - 1)   # previous head's j8
                        for j in sorted(jj for jj, rw in READY[h].items()
                                        if rw == w - 2):
                            emit_attnv(h, j)
                            if h == 3:
                                trq.append((j, w))
                    if h == 3:
                        if trq and trq[0][1] < w:
                            jj = trq.pop(0)[0]
                            emit_transpose(jj, 1)
                            unq.append((jj, w))
                        if unq and unq[0][1] < w:
                            emit_unify(unq.pop(0)[0])
            # tail: drain staged jobs, then the j8 chain
            while trq:
                jj = trq.pop(0)[0]
                emit_transpose(jj, 1)
                unq.append((jj, 99))
            if unq:
                emit_unify(unq.pop(0)[0])
            emit_attnv(3, NJ - 1)
            while unq:
                emit_unify(unq.pop(0)[0])
            emit_transpose(NJ - 1, 1)
            emit_unify(NJ - 1)

    nc.compile()
    return nc


def _get_nc():
    if "nc" not in _CACHED:
        _CACHED["nc"] = build_nc()
    return _CACHED["nc"]


def kernel(x, wk, bk, wq, bq, wv, bv, wo, bo):
    from concourse import bass_utils
    import ml_dtypes

    bf16 = ml_dtypes.bfloat16
    f32 = np.float32
    x = np.asarray(x, dtype=f32)
    wk, wq, wv, wo = (np.asarray(w, dtype=f32) for w in (wk, wq, wv, wo))
    bq, bv, bo = (np.asarray(b_, dtype=f32) for b_ in (bq, bv, bo))

    xb = x.reshape(B, C, N)
    fb = wo @ bv + bo
    bias = np.stack([bq[0:128], bq[128:256], fb[0:128], fb[128:256]],
                    axis=1).astype(f32)

    wkT, wqT, wvT, woT = wk.T, wq.T, wv.T, wo.T
    w3_cols = []
    for ct in range(2):
        rs = slice(ct * 128, (ct + 1) * 128)
        w3_cols += [wkT[rs, 0:128], wkT[rs, 128:256],
                    wqT[rs, 0:128], wqT[rs, 128:256], wvT[rs, :]]
    w3 = np.ascontiguousarray(np.concatenate(w3_cols, axis=1)).astype(bf16)
    woTh = np.ascontiguousarray(
        np.concatenate([woT[0:128, :], woT[128:256, :]], axis=1)).astype(bf16)

    def interleave(arr, chunks):
        blocks = []
        for c0, cw in chunks:
            for ct in range(2):
                blocks.append(arr[ct * 128:(ct + 1) * 128, c0:c0 + cw])
        return np.ascontiguousarray(np.concatenate(blocks, axis=1)).astype(bf16)

    XQCH = [(0, 512), (512, 512), (1024, 128)]
    nc = _get_nc()
    in_maps = []
    xh = {}
    for c in range(8):
        b, qh = c // 2, c % 2
        if b not in xh:
            xh[b] = interleave(xb[b], KCH)
        in_maps.append({
            "x": xh[b],
            "xq": interleave(xb[b][:, qh * M:(qh + 1) * M], XQCH),
            "w3": w3, "woT": woTh, "bias": bias,
        })
    global last_in_maps
    last_in_maps = in_maps
    res = bass_utils.run_bass_kernel_spmd(nc, in_maps, core_ids=list(range(8)))

    out = np.empty((B, C, N), dtype=f32)
    for c in range(8):
        b, qh = c // 2, c % 2
        out[b][:, qh * M:(qh + 1) * M] = res.results[c]["out"]
    return out.reshape(B, C, HW, HW)
